# revision 11
# baseline (speedup 1.0000x reference)
# Trainium2 Bass kernel for nn_BQQLinear (quantized bilinear linear layer).
#
# Math: the reference collapses exactly to
#     out[b, (j,m)] = quant8(x)[b, (k,n)] @ W[(k,n), (j,m)] + bias[(j,m)]
# where W folds the 1-bit-quantized Y/Z factors and the A-correction terms
# (see _fold_weights). W is a pure function of the small weight tensors ->
# folded on host at load time. The activation quant is elementwise host
# prep; the 2048x1024x1024 matmul + bias runs on the NeuronCores.
#
# Sharding: 4-way over batch x 2-way over output columns. Per core:
# [512, 1024] @ [1024, 512] + bias.
#
# PE strategy: fp8e4m3 DoubleRow matmuls (2 ifmap columns/cycle => 256-deep
# contraction per instruction at the same 216ns as a normal 512-wide
# matmul). W is split W ~= H + R with both terms e4m3 at the SAME scale,
# accumulated into the same PSUM bank; R is applied to 6 of the 8 k-tiles
# (the two k-tiles with the smallest residual energy are permuted into the
# first k-pair and skip R). 28 matmul instructions instead of 32, rel err
# ~1.44e-2 (vs 2e-2 gate).
#
# Schedule: DMA issues spread over scalar/sync/vector engines in parallel
# (each DIRECT2D costs ~650ns of sequencer time), first chunks small so
# the real matmul stream starts ~2us into the exec window, k-pair-outer /
# bank-inner order tracks the W stream, last two k-pairs bank-major so
# PSUM banks finish staggered and the DVE evict + out-DMA pipeline
# overlaps the matmul tail. A PE warm spinner keeps the clock ramped from
# window start until the first chunk lands.

import numpy as np
import ml_dtypes

import concourse.bacc as bacc
import concourse.mybir as mybir
import concourse.tile as tile
from concourse.bass import ts
from concourse.bass_utils import run_bass_kernel_spmd

N_CORES = 8
P = 128
KN = 1024                # k*n contraction dim
JM = 1024                # j*m output dim
B_TOT = 2048             # flattened batch
B_SHARDS = 4
C_SHARDS = 2
B_C = B_TOT // B_SHARDS      # 512 rows per core
JM_C = JM // C_SHARDS        # 512 cols per core
B_TILES = B_C // P           # 4
K_TILES = KN // P            # 8
KP = K_TILES // 2            # 4 k-pairs (DoubleRow contracts 2 k-tiles)
R_PAIRS = (1, 2, 3)          # k-pairs that get the residual term
QMAX = 127.0
F8_SAT = 112.0               # e4m3 grid point; q=+-127 maps here exactly
F8_DT = mybir.dt.float8e4
F8_NP = ml_dtypes.float8_e4m3
W_AMAX = 8.0                 # |W*gamma| target max
MM_DT = mybir.dt.float16
DR = mybir.MatmulPerfMode.DoubleRow
WARM_NS = [512] * 6 + [128] * 2   # spinner MM sizes
# chunking: x split (kp0, kp1-3); W split (kp0 H | kp1 H+R | kp2 | kp3).
# Each chunk is its own DRAM tensor so every DMA is fully contiguous.
XC0_KP = 1                   # k-pairs in first x chunk


def _fold_weights(Y_fp, Z_fp, A, dtype=np.float64):
    """Fold the quantized factorization into a single [KN, JM] weight."""
    Y = Y_fp.astype(dtype)
    Z = Z_fp.astype(dtype)
    Af = A.astype(dtype)
    p, j, k, m, l = Y.shape
    n = Z.shape[-1]

    Y_scale = np.mean(np.abs(Y), axis=(-2, -1), keepdims=True)
    Z_scale = np.mean(np.abs(Z), axis=(-2, -1), keepdims=True)
    Y_q = np.abs(Y_scale) * np.sign(Y)          # (p,j,k,m,l)
    Z_q = np.abs(Z_scale) * np.sign(Z)          # (p,j,k,l,n)

    # out1: sum_{p,l} A0 * Y_q * Z_q  -> [k,n,j,m]
    W = np.einsum('pjk,pjkml,pjkln->knjm', Af[..., 0], Y_q, Z_q, optimize=True)
    # out2: B_coef[j,k,m] = sum_p A1 * sum_l Y_q ; X enters via Sx (sum over n)
    B_coef = np.einsum('pjk,pjkm->jkm', Af[..., 1], Y_q.sum(-1))
    W += B_coef.transpose(1, 0, 2)[:, None, :, :]
    # out3: C_coef[j,k,n] = sum_p A2 * sum_l Z_q ; broadcast over m
    C_coef = np.einsum('pjk,pjkn->jkn', Af[..., 2], Z_q.sum(-2))
    W += C_coef.transpose(1, 2, 0)[:, :, :, None]
    # out4: D_coef[j,k] = sum_p A3 ; broadcast over n, m
    W += Af[..., 3].sum(0).T[:, None, :, None]
    return W.reshape(k * n, j * m)


def _build(inv_gamma):
    """Per-core Tile kernel: [B_C,KN] @ [KN,JM_C] + bias, evict on DVE."""
    nc = bacc.Bacc(
        "TRN2", target_bir_lowering=False, debug=False,
        enable_asserts=False, num_devices=N_CORES,
        enable_partition_id=False,
    )
    # x: [P, kp, 2, B_C] e4m3; W: chunks [H(kp0) | H+R per kp]
    xt0 = nc.dram_tensor("xt0", [P, XC0_KP * 2 * B_C], F8_DT,
                         kind="ExternalInput").ap()
    xt1 = nc.dram_tensor("xt1", [P, (KP - XC0_KP) * 2 * B_C], F8_DT,
                         kind="ExternalInput").ap()
    wt0 = nc.dram_tensor("wt0", [P, 2 * JM_C], F8_DT,
                         kind="ExternalInput").ap()
    wts = [nc.dram_tensor(f"wt{c}", [P, 4 * JM_C], F8_DT,
                          kind="ExternalInput").ap() for c in (1, 2, 3)]
    bi = nc.dram_tensor("bi", [P, JM_C], MM_DT, kind="ExternalInput").ap()
    out = nc.dram_tensor("out", [B_C, JM_C], MM_DT, kind="ExternalOutput").ap()

    out_t = out.rearrange("(bt p) j -> bt p j", p=P)

    with tile.TileContext(nc) as tc:
        with (
            tc.tile_pool(name="sb", bufs=1) as sb,
            tc.tile_pool(name="ps", bufs=1, space="PSUM") as ps,
        ):
            # --- warm-tile memset first: the PE spinner is gated on it ---
            warm_sb = sb.tile([P, 512], MM_DT, tag="warm")
            nc.gpsimd.memset(warm_sb[:, 0:1], 0.0)

            # --- input DMAs: issues spread over the two HWDGE rings,
            # ordered by when the data is needed ---
            x0_sb = sb.tile([P, XC0_KP, 2, B_C], F8_DT, tag="x0", name="x0")
            x1_sb = sb.tile([P, KP - XC0_KP, 2, B_C], F8_DT, tag="x1",
                            name="x1")
            w0_sb = sb.tile([P, 1, 2, JM_C], F8_DT, tag="w0", name="w0")
            w_sb = [sb.tile([P, 2, 2, JM_C], F8_DT, tag=f"w{c}", name=f"w{c}")
                    for c in (1, 2, 3)]
            bias_sb = sb.tile([P, JM_C], MM_DT, tag="bias")

            nc.scalar.dma_start(x0_sb[:], xt0.rearrange(
                "p (kp two b) -> p kp two b", kp=XC0_KP, two=2))
            nc.sync.dma_start(w0_sb[:], wt0.rearrange(
                "p (hr two j) -> p hr two j", hr=1, two=2))
            nc.scalar.dma_start(x1_sb[:], xt1.rearrange(
                "p (kp two b) -> p kp two b", kp=KP - XC0_KP, two=2))
            nc.sync.dma_start(w_sb[0][:], wts[0].rearrange(
                "p (hr two j) -> p hr two j", hr=2, two=2))
            nc.sync.dma_start(w_sb[1][:], wts[1].rearrange(
                "p (hr two j) -> p hr two j", hr=2, two=2))
            nc.scalar.dma_start(w_sb[2][:], wts[2].rearrange(
                "p (hr two j) -> p hr two j", hr=2, two=2))
            nc.scalar.dma_start(bias_sb[:], bi[:])

            # --- PE warm spinner (results unused): keeps the clock ramping
            # while the first chunks land
            warm_psum = ps.tile([1, 512], mybir.dt.float32, tag="pswarm")
            for n in WARM_NS:
                nc.tensor.matmul(
                    warm_psum[:, :n], lhsT=warm_sb[:, 0:1], rhs=warm_sb[:, :n],
                    start=True, stop=True,
                )

            psum = {
                bt: ps.tile([P, JM_C], mybir.dt.float32, tag=f"ps{bt}",
                            name=f"ps{bt}")
                for bt in range(B_TILES)
            }

            def x_ap(kp, bt):
                if kp < XC0_KP:
                    return x0_sb[:, kp, :, ts(bt, P)]
                return x1_sb[:, kp - XC0_KP, :, ts(bt, P)]

            def mm(kp, bt, hr, start, stop):
                rhs = w0_sb[:, 0] if kp == 0 else w_sb[kp - 1][:, hr]
                nc.tensor.matmul(
                    psum[bt][:], lhsT=x_ap(kp, bt), rhs=rhs,
                    start=start, stop=stop, perf_mode=DR,
                )

            # kp0 (H only) + kp1 (H,R): k-outer, bank-inner
            for bt in range(B_TILES):
                mm(0, bt, 0, True, False)
            for bt in range(B_TILES):
                mm(1, bt, 0, False, False)
                mm(1, bt, 1, False, False)
            # kp2+kp3: bank-major so banks finish staggered for the
            # evict + out-DMA pipeline
            NH = JM_C // 2
            o_sb = {}
            for bt in range(B_TILES):
                mm(2, bt, 0, False, False)
                mm(2, bt, 1, False, False)
                mm(3, bt, 0, False, False)
                mm(3, bt, 1, False, True)
                o_sb[bt] = sb.tile([P, JM_C], MM_DT, tag=f"o{bt}",
                                   name=f"o{bt}")
                if bt < B_TILES - 1:
                    # out = psum * inv_gamma + bias
                    nc.vector.scalar_tensor_tensor(
                        o_sb[bt][:], psum[bt][:], inv_gamma, bias_sb[:],
                        mybir.AluOpType.mult, mybir.AluOpType.add,
                    )
                    eng = nc.sync if bt % 2 == 0 else nc.scalar
                    eng.dma_start(out_t[bt][:], o_sb[bt][:])
                else:
                    # last bank: column-split evict (DVE time ~ free size),
                    # then partition-split DMAs (DIRECT2D cost ~ descriptor
                    # count) on both rings in parallel
                    for h in range(2):
                        nc.vector.scalar_tensor_tensor(
                            o_sb[bt][:, ts(h, NH)], psum[bt][:, ts(h, NH)],
                            inv_gamma, bias_sb[:, ts(h, NH)],
                            mybir.AluOpType.mult, mybir.AluOpType.add,
                        )
                        nc.sync.dma_start(
                            out_t[bt][0:64, ts(h, NH)],
                            o_sb[bt][0:64, ts(h, NH)])
                        nc.scalar.dma_start(
                            out_t[bt][64:128, ts(h, NH)],
                            o_sb[bt][64:128, ts(h, NH)])

            # keep the warm matmuls live (guard against DCE)
            sink = sb.tile([1, 1], mybir.dt.float32, tag="sink")
            nc.vector.tensor_copy(out=sink[:], in_=warm_psum[0:1, 0:1])

    nc.compile()
    return nc


def _prepare_inputs(x, Y_fp, Z_fp, A, bias, act_scale):
    s = max(abs(float(np.asarray(act_scale).reshape(-1)[0])), 1e-8)
    # activation quant -> integer codes, then e4m3 with saturation at F8_SAT
    q = np.clip(np.rint(x.astype(np.float32).reshape(B_TOT, KN)
                        / np.float32(s)), -QMAX, QMAX)
    qf8 = (q * np.float32(F8_SAT / QMAX)).astype(F8_NP)

    # fold everything else into W; quantize W*gamma ~= H + R (both e4m3,
    # same scale -> accumulate in one PSUM group)
    W = _fold_weights(Y_fp, Z_fp, A)
    W_s = W * (s * QMAX / F8_SAT)
    gamma = W_AMAX / np.abs(W_s).max()
    Wg = (W_s * gamma).astype(np.float32)
    H = Wg.astype(F8_NP)
    R = (Wg - H.astype(np.float32)).astype(F8_NP)
    inv_gamma = float(1.0 / np.float32(gamma))

    # permute k-tiles so the two lowest-residual tiles form k-pair 0
    # (which skips R)
    resid = ((Wg - H.astype(np.float32)) ** 2).reshape(K_TILES, P, JM)
    order = np.argsort(resid.sum(axis=(1, 2)))            # ascending
    H8 = H.reshape(K_TILES, P, JM)[order]                 # [8,128,JM]
    R8 = R.reshape(K_TILES, P, JM)[order]
    qT = np.ascontiguousarray(qf8.T).reshape(K_TILES, P, B_TOT)[order]

    bias16 = np.asarray(bias, dtype=np.float16)

    in_maps = []
    for core in range(N_CORES):
        bs, js = core % B_SHARDS, core // B_SHARDS
        xk = qT[:, :, bs * B_C:(bs + 1) * B_C]            # [8,128,B_C]
        xp = xk.reshape(KP, 2, P, B_C)
        x0 = np.ascontiguousarray(
            xp[:XC0_KP].transpose(2, 0, 1, 3)).reshape(P, XC0_KP * 2 * B_C)
        x1 = np.ascontiguousarray(
            xp[XC0_KP:].transpose(2, 0, 1, 3)
        ).reshape(P, (KP - XC0_KP) * 2 * B_C)
        Hc = H8[:, :, js * JM_C:(js + 1) * JM_C]          # [8,128,JM_C]
        Rc = R8[:, :, js * JM_C:(js + 1) * JM_C]
        Hp = Hc.reshape(KP, 2, P, JM_C)
        Rp = Rc.reshape(KP, 2, P, JM_C)
        im = {
            "xt0": x0, "xt1": x1,
            "wt0": np.ascontiguousarray(
                Hp[0].transpose(1, 0, 2)).reshape(P, 2 * JM_C),
        }
        for i, kp in enumerate(R_PAIRS):
            blk = np.stack([Hp[kp], Rp[kp]], axis=0)      # [2(hr),2,P,JM_C]
            im[f"wt{i + 1}"] = np.ascontiguousarray(
                blk.transpose(2, 0, 1, 3)).reshape(P, 4 * JM_C)
        im["bi"] = np.ascontiguousarray(
            np.broadcast_to(bias16[js * JM_C:(js + 1) * JM_C][None, :],
                            (P, JM_C)))
        in_maps.append(im)
    return in_maps, inv_gamma


def kernel_run(x, Y_fp, Z_fp, A, bias, act_scale, trace=False, **spmd_kwargs):
    """Build + run on 8 NeuronCores; returns (out, BassKernelResults)."""
    in_maps, inv_gamma = _prepare_inputs(x, Y_fp, Z_fp, A, bias, act_scale)
    nc = _build(inv_gamma)
    res = run_bass_kernel_spmd(
        nc, in_maps, core_ids=list(range(N_CORES)), trace=trace, **spmd_kwargs
    )
    full = np.empty((B_TOT, JM), dtype=np.float32)
    for core in range(N_CORES):
        bs, js = core % B_SHARDS, core // B_SHARDS
        full[bs * B_C:(bs + 1) * B_C, js * JM_C:(js + 1) * JM_C] = (
            res.results[core]["out"].astype(np.float32)
        )
    out = full.reshape(x.shape[0], x.shape[1], JM).astype(x.dtype, copy=False)
    return out, res


def kernel(x, Y_fp, Z_fp, A, bias, act_scale):
    x = np.asarray(x)
    Y_fp = np.asarray(Y_fp)
    Z_fp = np.asarray(Z_fp)
    A = np.asarray(A)
    bias = np.asarray(bias)
    act_scale = np.asarray(act_scale)
    out, _ = kernel_run(x, Y_fp, Z_fp, A, bias, act_scale, trace=False)
    return out


# revision 16
# speedup vs baseline: 1.0954x; 1.0954x over previous
# Trainium2 Bass kernel for nn_BQQLinear (quantized bilinear linear layer).
#
# Math: the reference collapses exactly to
#     out[b, (j,m)] = quant8(x)[b, (k,n)] @ W[(k,n), (j,m)] + bias[(j,m)]
# where W folds the 1-bit-quantized Y/Z factors and the A-correction terms
# (see _fold_weights). W is a pure function of the small weight tensors ->
# folded on host at load time. The activation quant is elementwise host
# prep; the 2048x1024x1024 matmul + bias runs on the NeuronCores.
#
# Sharding: 4-way over batch x 2-way over output columns. Per core:
# [512, 1024] @ [1024, 512] + bias.
#
# PE strategy: fp8e4m3 DoubleRow matmuls (2 ifmap columns/cycle => 256-deep
# contraction per instruction at the same 216ns as a normal 512-wide
# matmul). W is split W ~= H + R with both terms e4m3 at the SAME scale,
# accumulated into the same PSUM bank; R is applied to 6 of the 8 k-tiles
# (the two k-tiles with the smallest residual energy are permuted into the
# first k-pair and skip R). 28 matmul instructions instead of 32, rel err
# ~1.44e-2 (vs 2e-2 gate).
#
# Schedule: DMA issues spread over scalar/sync/vector engines in parallel
# (each DIRECT2D costs ~650ns of sequencer time), first chunks small so
# the real matmul stream starts ~2us into the exec window, k-pair-outer /
# bank-inner order tracks the W stream, last two k-pairs bank-major so
# PSUM banks finish staggered and the DVE evict + out-DMA pipeline
# overlaps the matmul tail. A PE warm spinner keeps the clock ramped from
# window start until the first chunk lands.

import numpy as np
import ml_dtypes

import concourse.bacc as bacc
import concourse.mybir as mybir
import concourse.tile as tile
from concourse.bass import ts
from concourse.bass_utils import run_bass_kernel_spmd

N_CORES = 8
P = 128
KN = 1024                # k*n contraction dim
JM = 1024                # j*m output dim
B_TOT = 2048             # flattened batch
B_SHARDS = 4
C_SHARDS = 2
B_C = B_TOT // B_SHARDS      # 512 rows per core
JM_C = JM // C_SHARDS        # 512 cols per core
B_TILES = B_C // P           # 4
K_TILES = KN // P            # 8
KP = K_TILES // 2            # 4 k-pairs (DoubleRow contracts 2 k-tiles)
R_PAIRS = (1, 2, 3)          # k-pairs that get the residual term
QMAX = 127.0
F8_SAT = 112.0               # e4m3 grid point; q=+-127 maps here exactly
F8_DT = mybir.dt.float8e4
F8_NP = ml_dtypes.float8_e4m3
W_AMAX = 8.0                 # |W*gamma| target max
MM_DT = mybir.dt.float16
DR = mybir.MatmulPerfMode.DoubleRow
WARM_NS = [512] * 5 + [128] * 2   # spinner MM sizes
# chunking: per-k-pair x and W tensors, DMAs issued in need order
# alternating between the two HWDGE rings (sync: W, scalar: x) so the
# shared queues serve chunks in consumption order. Each chunk is its own
# DRAM tensor so every DMA is fully contiguous.


def _fold_weights(Y_fp, Z_fp, A, dtype=np.float64):
    """Fold the quantized factorization into a single [KN, JM] weight."""
    Y = Y_fp.astype(dtype)
    Z = Z_fp.astype(dtype)
    Af = A.astype(dtype)
    p, j, k, m, l = Y.shape
    n = Z.shape[-1]

    Y_scale = np.mean(np.abs(Y), axis=(-2, -1), keepdims=True)
    Z_scale = np.mean(np.abs(Z), axis=(-2, -1), keepdims=True)
    Y_q = np.abs(Y_scale) * np.sign(Y)          # (p,j,k,m,l)
    Z_q = np.abs(Z_scale) * np.sign(Z)          # (p,j,k,l,n)

    # out1: sum_{p,l} A0 * Y_q * Z_q  -> [k,n,j,m]
    W = np.einsum('pjk,pjkml,pjkln->knjm', Af[..., 0], Y_q, Z_q, optimize=True)
    # out2: B_coef[j,k,m] = sum_p A1 * sum_l Y_q ; X enters via Sx (sum over n)
    B_coef = np.einsum('pjk,pjkm->jkm', Af[..., 1], Y_q.sum(-1))
    W += B_coef.transpose(1, 0, 2)[:, None, :, :]
    # out3: C_coef[j,k,n] = sum_p A2 * sum_l Z_q ; broadcast over m
    C_coef = np.einsum('pjk,pjkn->jkn', Af[..., 2], Z_q.sum(-2))
    W += C_coef.transpose(1, 2, 0)[:, :, :, None]
    # out4: D_coef[j,k] = sum_p A3 ; broadcast over n, m
    W += Af[..., 3].sum(0).T[:, None, :, None]
    return W.reshape(k * n, j * m)


def _build(inv_gamma):
    """Per-core Tile kernel: [B_C,KN] @ [KN,JM_C] + bias, evict on DVE."""
    nc = bacc.Bacc(
        "TRN2", target_bir_lowering=False, debug=False,
        enable_asserts=False, num_devices=N_CORES,
        enable_partition_id=False,
    )
    # per-k-pair tensors: x [P, 2, B_C]; W kp0 = H only, kp1-3 = H+R
    xts = [nc.dram_tensor(f"xt{kp}", [P, 2 * B_C], F8_DT,
                          kind="ExternalInput").ap() for kp in range(KP)]
    wt0 = nc.dram_tensor("wt0", [P, 2 * JM_C], F8_DT,
                         kind="ExternalInput").ap()
    wts = [nc.dram_tensor(f"wt{c}", [P, 4 * JM_C], F8_DT,
                          kind="ExternalInput").ap() for c in (1, 2, 3)]
    bi = nc.dram_tensor("bi", [P, JM_C], MM_DT, kind="ExternalInput").ap()
    out = nc.dram_tensor("out", [B_C, JM_C], MM_DT, kind="ExternalOutput").ap()

    out_t = out.rearrange("(bt p) j -> bt p j", p=P)

    with tile.TileContext(nc) as tc:
        with (
            tc.tile_pool(name="sb", bufs=1) as sb,
            tc.tile_pool(name="ps", bufs=1, space="PSUM") as ps,
        ):
            # --- warm-tile memset first: the PE spinner is gated on it ---
            warm_sb = sb.tile([P, 512], MM_DT, tag="warm")
            nc.gpsimd.memset(warm_sb[:, 0:1], 0.0)

            # --- input DMAs: need-order, alternating rings ---
            x_sb = [sb.tile([P, 2, B_C], F8_DT, tag=f"x{kp}", name=f"x{kp}")
                    for kp in range(KP)]
            w0_sb = sb.tile([P, 1, 2, JM_C], F8_DT, tag="w0", name="w0")
            w_sb = [sb.tile([P, 2, 2, JM_C], F8_DT, tag=f"w{c}", name=f"w{c}")
                    for c in (1, 2, 3)]
            bias_sb = sb.tile([P, JM_C], MM_DT, tag="bias")

            nc.sync.dma_start(w0_sb[:], wt0.rearrange(
                "p (hr two j) -> p hr two j", hr=1, two=2))
            nc.scalar.dma_start(x_sb[0][:], xts[0].rearrange(
                "p (two b) -> p two b", two=2))
            for kp in (1, 2, 3):
                nc.sync.dma_start(w_sb[kp - 1][:], wts[kp - 1].rearrange(
                    "p (hr two j) -> p hr two j", hr=2, two=2))
                nc.scalar.dma_start(x_sb[kp][:], xts[kp].rearrange(
                    "p (two b) -> p two b", two=2))
            nc.scalar.dma_start(bias_sb[:], bi[:])

            # --- PE warm spinner (results unused): keeps the clock ramping
            # while the first chunks land
            warm_psum = ps.tile([1, 512], mybir.dt.float32, tag="pswarm")
            for n in WARM_NS:
                nc.tensor.matmul(
                    warm_psum[:, :n], lhsT=warm_sb[:, 0:1], rhs=warm_sb[:, :n],
                    start=True, stop=True,
                )

            psum = {
                bt: ps.tile([P, JM_C], mybir.dt.float32, tag=f"ps{bt}",
                            name=f"ps{bt}")
                for bt in range(B_TILES)
            }

            def x_ap(kp, bt):
                return x_sb[kp][:, :, ts(bt, P)]

            def mm(kp, bt, hr, start, stop):
                rhs = w0_sb[:, 0] if kp == 0 else w_sb[kp - 1][:, hr]
                nc.tensor.matmul(
                    psum[bt][:], lhsT=x_ap(kp, bt), rhs=rhs,
                    start=start, stop=stop, perf_mode=DR,
                )

            # kp0 (H only) + kp1 (H,R): k-outer, bank-inner
            for bt in range(B_TILES):
                mm(0, bt, 0, True, False)
            for bt in range(B_TILES):
                mm(1, bt, 0, False, False)
                mm(1, bt, 1, False, False)
            # kp2+kp3: bank-major so banks finish staggered for the
            # evict + out-DMA pipeline
            NH = JM_C // 2
            o_sb = {}
            for bt in range(B_TILES):
                mm(2, bt, 0, False, False)
                mm(2, bt, 1, False, False)
                mm(3, bt, 0, False, False)
                mm(3, bt, 1, False, True)
                o_sb[bt] = sb.tile([P, JM_C], MM_DT, tag=f"o{bt}",
                                   name=f"o{bt}")
                if bt < B_TILES - 1:
                    # out = psum * inv_gamma + bias
                    nc.vector.scalar_tensor_tensor(
                        o_sb[bt][:], psum[bt][:], inv_gamma, bias_sb[:],
                        mybir.AluOpType.mult, mybir.AluOpType.add,
                    )
                    eng = nc.sync if bt % 2 == 0 else nc.scalar
                    eng.dma_start(out_t[bt][:], o_sb[bt][:])
                else:
                    # last bank: column-split evict (DVE time ~ free size),
                    # then partition-split DMAs (DIRECT2D cost ~ descriptor
                    # count) on both rings in parallel
                    for h in range(2):
                        nc.vector.scalar_tensor_tensor(
                            o_sb[bt][:, ts(h, NH)], psum[bt][:, ts(h, NH)],
                            inv_gamma, bias_sb[:, ts(h, NH)],
                            mybir.AluOpType.mult, mybir.AluOpType.add,
                        )
                        nc.sync.dma_start(
                            out_t[bt][0:64, ts(h, NH)],
                            o_sb[bt][0:64, ts(h, NH)])
                        nc.scalar.dma_start(
                            out_t[bt][64:128, ts(h, NH)],
                            o_sb[bt][64:128, ts(h, NH)])

            # keep the warm matmuls live (guard against DCE)
            sink = sb.tile([1, 1], mybir.dt.float32, tag="sink")
            nc.vector.tensor_copy(out=sink[:], in_=warm_psum[0:1, 0:1])

    nc.compile()
    return nc


def _prepare_inputs(x, Y_fp, Z_fp, A, bias, act_scale):
    s = max(abs(float(np.asarray(act_scale).reshape(-1)[0])), 1e-8)
    # activation quant -> integer codes, then e4m3 with saturation at F8_SAT
    q = np.clip(np.rint(x.astype(np.float32).reshape(B_TOT, KN)
                        / np.float32(s)), -QMAX, QMAX)
    qf8 = (q * np.float32(F8_SAT / QMAX)).astype(F8_NP)

    # fold everything else into W; quantize W*gamma ~= H + R (both e4m3,
    # same scale -> accumulate in one PSUM group)
    W = _fold_weights(Y_fp, Z_fp, A)
    W_s = W * (s * QMAX / F8_SAT)
    gamma = W_AMAX / np.abs(W_s).max()
    Wg = (W_s * gamma).astype(np.float32)
    H = Wg.astype(F8_NP)
    R = (Wg - H.astype(np.float32)).astype(F8_NP)
    inv_gamma = float(1.0 / np.float32(gamma))

    # permute k-tiles so the two lowest-residual tiles form k-pair 0
    # (which skips R)
    resid = ((Wg - H.astype(np.float32)) ** 2).reshape(K_TILES, P, JM)
    order = np.argsort(resid.sum(axis=(1, 2)))            # ascending
    H8 = H.reshape(K_TILES, P, JM)[order]                 # [8,128,JM]
    R8 = R.reshape(K_TILES, P, JM)[order]
    qT = np.ascontiguousarray(qf8.T).reshape(K_TILES, P, B_TOT)[order]

    bias16 = np.asarray(bias, dtype=np.float16)

    in_maps = []
    for core in range(N_CORES):
        bs, js = core % B_SHARDS, core // B_SHARDS
        xk = qT[:, :, bs * B_C:(bs + 1) * B_C]            # [8,128,B_C]
        xp = xk.reshape(KP, 2, P, B_C)
        Hc = H8[:, :, js * JM_C:(js + 1) * JM_C]          # [8,128,JM_C]
        Rc = R8[:, :, js * JM_C:(js + 1) * JM_C]
        Hp = Hc.reshape(KP, 2, P, JM_C)
        Rp = Rc.reshape(KP, 2, P, JM_C)
        im = {
            "wt0": np.ascontiguousarray(
                Hp[0].transpose(1, 0, 2)).reshape(P, 2 * JM_C),
        }
        for kp in range(KP):
            im[f"xt{kp}"] = np.ascontiguousarray(
                xp[kp].transpose(1, 0, 2)).reshape(P, 2 * B_C)
        for i, kp in enumerate(R_PAIRS):
            blk = np.stack([Hp[kp], Rp[kp]], axis=0)      # [2(hr),2,P,JM_C]
            im[f"wt{i + 1}"] = np.ascontiguousarray(
                blk.transpose(2, 0, 1, 3)).reshape(P, 4 * JM_C)
        im["bi"] = np.ascontiguousarray(
            np.broadcast_to(bias16[js * JM_C:(js + 1) * JM_C][None, :],
                            (P, JM_C)))
        in_maps.append(im)
    return in_maps, inv_gamma


def kernel_run(x, Y_fp, Z_fp, A, bias, act_scale, trace=False, **spmd_kwargs):
    """Build + run on 8 NeuronCores; returns (out, BassKernelResults)."""
    in_maps, inv_gamma = _prepare_inputs(x, Y_fp, Z_fp, A, bias, act_scale)
    nc = _build(inv_gamma)
    res = run_bass_kernel_spmd(
        nc, in_maps, core_ids=list(range(N_CORES)), trace=trace, **spmd_kwargs
    )
    full = np.empty((B_TOT, JM), dtype=np.float32)
    for core in range(N_CORES):
        bs, js = core % B_SHARDS, core // B_SHARDS
        full[bs * B_C:(bs + 1) * B_C, js * JM_C:(js + 1) * JM_C] = (
            res.results[core]["out"].astype(np.float32)
        )
    out = full.reshape(x.shape[0], x.shape[1], JM).astype(x.dtype, copy=False)
    return out, res


def kernel(x, Y_fp, Z_fp, A, bias, act_scale):
    x = np.asarray(x)
    Y_fp = np.asarray(Y_fp)
    Z_fp = np.asarray(Z_fp)
    A = np.asarray(A)
    bias = np.asarray(bias)
    act_scale = np.asarray(act_scale)
    out, _ = kernel_run(x, Y_fp, Z_fp, A, bias, act_scale, trace=False)
    return out


# revision 23
# speedup vs baseline: 1.1828x; 1.0798x over previous
# Trainium2 Bass kernel for nn_BQQLinear (quantized bilinear linear layer).
#
# Math: the reference collapses exactly to
#     out[b, (j,m)] = quant8(x)[b, (k,n)] @ W[(k,n), (j,m)] + bias[(j,m)]
# where W folds the 1-bit-quantized Y/Z factors and the A-correction terms
# (see _fold_weights). W is a pure function of the small weight tensors ->
# folded on host at load time. The activation quant is elementwise host
# prep; the 2048x1024x1024 matmul + bias runs on the NeuronCores.
#
# Sharding: 4-way over batch x 2-way over output columns. Per core:
# [512, 1024] @ [1024, 512] + bias.
#
# PE strategy: fp8e4m3 DoubleRow matmuls (2 ifmap columns/cycle => 256-deep
# contraction per instruction at the same 216ns as a normal 512-wide
# matmul). W is split W ~= H + R with both terms e4m3 at the SAME scale,
# accumulated into the same PSUM bank; R is applied to 6 of the 8 k-tiles
# (the two k-tiles with the smallest residual energy are permuted into the
# first k-pair and skip R). 28 matmul instructions instead of 32, rel err
# ~1.44e-2 (vs 2e-2 gate).
#
# Schedule: DMA issues spread over scalar/sync/vector engines in parallel
# (each DIRECT2D costs ~650ns of sequencer time), first chunks small so
# the real matmul stream starts ~2us into the exec window, k-pair-outer /
# bank-inner order tracks the W stream, last two k-pairs bank-major so
# PSUM banks finish staggered and the DVE evict + out-DMA pipeline
# overlaps the matmul tail. A PE warm spinner keeps the clock ramped from
# window start until the first chunk lands.

import numpy as np
import ml_dtypes

import concourse.bacc as bacc
import concourse.mybir as mybir
import concourse.tile as tile
from concourse.bass import ts
from concourse.bass_utils import run_bass_kernel_spmd

N_CORES = 8
P = 128
KN = 1024                # k*n contraction dim
JM = 1024                # j*m output dim
B_TOT = 2048             # flattened batch
B_SHARDS = 4
C_SHARDS = 2
B_C = B_TOT // B_SHARDS      # 512 rows per core
JM_C = JM // C_SHARDS        # 512 cols per core
B_TILES = B_C // P           # 4
K_TILES = KN // P            # 8
KP = K_TILES // 2            # 4 k-pairs (DoubleRow contracts 2 k-tiles)
R_PAIRS = (0, 1, 2)          # k-pairs that get the residual term
QMAX = 127.0
F8_SAT = 112.0               # e4m3 grid point; q=+-127 maps here exactly
F8_DT = mybir.dt.float8e4
F8_NP = ml_dtypes.float8_e4m3
W_AMAX = 8.0                 # |W*gamma| target max
MM_DT = mybir.dt.float16
DR = mybir.MatmulPerfMode.DoubleRow
WARM_NS = [512] * 6 + [128] * 2   # spinner MM sizes
# chunking: per-k-pair x and W tensors, DMAs issued in need order
# alternating between the two HWDGE rings (sync: W, scalar: x) so the
# shared queues serve chunks in consumption order. Each chunk is its own
# DRAM tensor so every DMA is fully contiguous.


def _fold_weights(Y_fp, Z_fp, A, dtype=np.float64):
    """Fold the quantized factorization into a single [KN, JM] weight."""
    Y = Y_fp.astype(dtype)
    Z = Z_fp.astype(dtype)
    Af = A.astype(dtype)
    p, j, k, m, l = Y.shape
    n = Z.shape[-1]

    Y_scale = np.mean(np.abs(Y), axis=(-2, -1), keepdims=True)
    Z_scale = np.mean(np.abs(Z), axis=(-2, -1), keepdims=True)
    Y_q = np.abs(Y_scale) * np.sign(Y)          # (p,j,k,m,l)
    Z_q = np.abs(Z_scale) * np.sign(Z)          # (p,j,k,l,n)

    # out1: sum_{p,l} A0 * Y_q * Z_q  -> [k,n,j,m]
    W = np.einsum('pjk,pjkml,pjkln->knjm', Af[..., 0], Y_q, Z_q, optimize=True)
    # out2: B_coef[j,k,m] = sum_p A1 * sum_l Y_q ; X enters via Sx (sum over n)
    B_coef = np.einsum('pjk,pjkm->jkm', Af[..., 1], Y_q.sum(-1))
    W += B_coef.transpose(1, 0, 2)[:, None, :, :]
    # out3: C_coef[j,k,n] = sum_p A2 * sum_l Z_q ; broadcast over m
    C_coef = np.einsum('pjk,pjkn->jkn', Af[..., 2], Z_q.sum(-2))
    W += C_coef.transpose(1, 2, 0)[:, :, :, None]
    # out4: D_coef[j,k] = sum_p A3 ; broadcast over n, m
    W += Af[..., 3].sum(0).T[:, None, :, None]
    return W.reshape(k * n, j * m)


def _build(inv_gamma):
    """Per-core Tile kernel: [B_C,KN] @ [KN,JM_C] + bias, evict on DVE."""
    nc = bacc.Bacc(
        "TRN2", target_bir_lowering=False, debug=False,
        enable_asserts=False, num_devices=N_CORES,
        enable_partition_id=False,
    )
    # per-k-pair tensors: x [P, 2, B_C]; W kp0 split H/R (small first
    # chunks), kp1-2 = H+R, kp3 = H only (R skipped there)
    xts = [nc.dram_tensor(f"xt{kp}", [P, 2 * B_C], F8_DT,
                          kind="ExternalInput").ap() for kp in range(KP)]
    wh0 = nc.dram_tensor("wh0", [P, 2 * JM_C], F8_DT,
                         kind="ExternalInput").ap()
    wr0 = nc.dram_tensor("wr0", [P, 2 * JM_C], F8_DT,
                         kind="ExternalInput").ap()
    wts = [nc.dram_tensor(f"wt{c}", [P, 4 * JM_C], F8_DT,
                          kind="ExternalInput").ap() for c in (1, 2)]
    wh3 = nc.dram_tensor("wh3", [P, 2 * JM_C], F8_DT,
                         kind="ExternalInput").ap()
    bi = nc.dram_tensor("bi", [P, JM_C], MM_DT, kind="ExternalInput").ap()
    out = nc.dram_tensor("out", [B_C, JM_C], MM_DT, kind="ExternalOutput").ap()

    out_t = out.rearrange("(bt p) j -> bt p j", p=P)

    with tile.TileContext(nc) as tc:
        with (
            tc.tile_pool(name="sb", bufs=1) as sb,
            tc.tile_pool(name="ps", bufs=1, space="PSUM") as ps,
        ):
            # --- warm-tile memset first: the PE spinner is gated on it ---
            warm_sb = sb.tile([P, 512], MM_DT, tag="warm")
            nc.gpsimd.memset(warm_sb[:, 0:1], 0.0)

            # --- input DMAs: need-order, alternating rings ---
            x_sb = [sb.tile([P, 2, B_C], F8_DT, tag=f"x{kp}", name=f"x{kp}")
                    for kp in range(KP)]
            wh0_sb = sb.tile([P, 2, JM_C], F8_DT, tag="wh0", name="wh0")
            wr0_sb = sb.tile([P, 2, JM_C], F8_DT, tag="wr0", name="wr0")
            w_sb = [sb.tile([P, 2, 2, JM_C], F8_DT, tag=f"w{c}", name=f"w{c}")
                    for c in (1, 2)]
            wh3_sb = sb.tile([P, 2, JM_C], F8_DT, tag="wh3", name="wh3")
            bias_sb = sb.tile([P, JM_C], MM_DT, tag="bias")

            two = lambda t: t.rearrange("p (two j) -> p two j", two=2)
            hr2 = lambda t: t.rearrange("p (hr two j) -> p hr two j",
                                        hr=2, two=2)
            nc.sync.dma_start(wh0_sb[:], two(wh0))
            nc.scalar.dma_start(x_sb[0][:], two(xts[0]))
            nc.sync.dma_start(wr0_sb[:], two(wr0))
            nc.scalar.dma_start(x_sb[1][:], two(xts[1]))
            nc.sync.dma_start(w_sb[0][:], hr2(wts[0]))
            nc.scalar.dma_start(x_sb[2][:], two(xts[2]))
            nc.sync.dma_start(w_sb[1][:], hr2(wts[1]))
            nc.scalar.dma_start(x_sb[3][:], two(xts[3]))
            nc.sync.dma_start(wh3_sb[:], two(wh3))
            nc.scalar.dma_start(bias_sb[:], bi[:])

            # --- PE warm spinner (results unused): keeps the clock ramping
            # while the first chunks land
            warm_psum = ps.tile([1, 512], mybir.dt.float32, tag="pswarm")
            for n in WARM_NS:
                nc.tensor.matmul(
                    warm_psum[:, :n], lhsT=warm_sb[:, 0:1], rhs=warm_sb[:, :n],
                    start=True, stop=True,
                )

            psum = {
                bt: ps.tile([P, JM_C], mybir.dt.float32, tag=f"ps{bt}",
                            name=f"ps{bt}")
                for bt in range(B_TILES)
            }

            def mm(kp, bt, hr, start, stop):
                if kp == 0:
                    rhs = wh0_sb[:] if hr == 0 else wr0_sb[:]
                elif kp == 3:
                    rhs = wh3_sb[:]
                else:
                    rhs = w_sb[kp - 1][:, hr]
                nc.tensor.matmul(
                    psum[bt][:], lhsT=x_sb[kp][:, :, ts(bt, P)], rhs=rhs,
                    start=start, stop=stop, perf_mode=DR,
                )

            # kp0 H, kp0 R, kp1 (H,R): k-outer, bank-inner -> tracks the
            # chunk stream
            for bt in range(B_TILES):
                mm(0, bt, 0, True, False)
            for bt in range(B_TILES):
                mm(0, bt, 1, False, False)
            for bt in range(B_TILES):
                mm(1, bt, 0, False, False)
                mm(1, bt, 1, False, False)
            # kp2+kp3: bank-major so banks finish staggered for the
            # evict + out-DMA pipeline
            NH = JM_C // 2
            o_sb = {}
            for bt in range(B_TILES):
                mm(2, bt, 0, False, False)
                mm(2, bt, 1, False, False)
                mm(3, bt, 0, False, True)
                o_sb[bt] = sb.tile([P, JM_C], MM_DT, tag=f"o{bt}",
                                   name=f"o{bt}")
                if bt < B_TILES - 1:
                    # out = psum * inv_gamma + bias
                    nc.vector.scalar_tensor_tensor(
                        o_sb[bt][:], psum[bt][:], inv_gamma, bias_sb[:],
                        mybir.AluOpType.mult, mybir.AluOpType.add,
                    )
                    eng = nc.sync if bt % 2 == 0 else nc.scalar
                    eng.dma_start(out_t[bt][:], o_sb[bt][:])
                else:
                    # last bank: column-split evict (DVE time ~ free size),
                    # then partition-split DMAs (DIRECT2D cost ~ descriptor
                    # count) on both rings in parallel
                    for h in range(2):
                        nc.vector.scalar_tensor_tensor(
                            o_sb[bt][:, ts(h, NH)], psum[bt][:, ts(h, NH)],
                            inv_gamma, bias_sb[:, ts(h, NH)],
                            mybir.AluOpType.mult, mybir.AluOpType.add,
                        )
                        nc.sync.dma_start(
                            out_t[bt][0:64, ts(h, NH)],
                            o_sb[bt][0:64, ts(h, NH)])
                        nc.scalar.dma_start(
                            out_t[bt][64:128, ts(h, NH)],
                            o_sb[bt][64:128, ts(h, NH)])

            # keep the warm matmuls live (guard against DCE)
            sink = sb.tile([1, 1], mybir.dt.float32, tag="sink")
            nc.vector.tensor_copy(out=sink[:], in_=warm_psum[0:1, 0:1])

    nc.compile()
    return nc


def _prepare_inputs(x, Y_fp, Z_fp, A, bias, act_scale):
    s = max(abs(float(np.asarray(act_scale).reshape(-1)[0])), 1e-8)
    # activation quant -> integer codes, then e4m3 with saturation at F8_SAT
    q = np.clip(np.rint(x.astype(np.float32).reshape(B_TOT, KN)
                        / np.float32(s)), -QMAX, QMAX)
    qf8 = (q * np.float32(F8_SAT / QMAX)).astype(F8_NP)

    # fold everything else into W; quantize W*gamma ~= H + R (both e4m3,
    # same scale -> accumulate in one PSUM group)
    W = _fold_weights(Y_fp, Z_fp, A)
    W_s = W * (s * QMAX / F8_SAT)
    gamma = W_AMAX / np.abs(W_s).max()
    Wg = (W_s * gamma).astype(np.float32)
    H = Wg.astype(F8_NP)
    R = (Wg - H.astype(np.float32)).astype(F8_NP)
    inv_gamma = float(1.0 / np.float32(gamma))

    # permute k-tiles so the two lowest-residual tiles form the last
    # k-pair (which skips R)
    resid = ((Wg - H.astype(np.float32)) ** 2).reshape(K_TILES, P, JM)
    order = np.argsort(resid.sum(axis=(1, 2)))            # ascending
    perm = np.concatenate([np.sort(order[2:]), order[:2]])
    H8 = H.reshape(K_TILES, P, JM)[perm]                  # [8,128,JM]
    R8 = R.reshape(K_TILES, P, JM)[perm]
    qT = np.ascontiguousarray(qf8.T).reshape(K_TILES, P, B_TOT)[perm]

    bias16 = np.asarray(bias, dtype=np.float16)

    in_maps = []
    for core in range(N_CORES):
        bs, js = core % B_SHARDS, core // B_SHARDS
        xk = qT[:, :, bs * B_C:(bs + 1) * B_C]            # [8,128,B_C]
        xp = xk.reshape(KP, 2, P, B_C)
        Hc = H8[:, :, js * JM_C:(js + 1) * JM_C]          # [8,128,JM_C]
        Rc = R8[:, :, js * JM_C:(js + 1) * JM_C]
        Hp = Hc.reshape(KP, 2, P, JM_C)
        Rp = Rc.reshape(KP, 2, P, JM_C)

        def pt(a):                                        # [2,P,F] -> [P,2F]
            return np.ascontiguousarray(a.transpose(1, 0, 2)).reshape(
                P, -1)

        im = {
            "wh0": pt(Hp[0]),
            "wr0": pt(Rp[0]),
            "wh3": pt(Hp[3]),
        }
        for kp in range(KP):
            im[f"xt{kp}"] = pt(xp[kp])
        for kp in (1, 2):
            blk = np.stack([Hp[kp], Rp[kp]], axis=0)      # [2(hr),2,P,JM_C]
            im[f"wt{kp}"] = np.ascontiguousarray(
                blk.transpose(2, 0, 1, 3)).reshape(P, 4 * JM_C)
        im["bi"] = np.ascontiguousarray(
            np.broadcast_to(bias16[js * JM_C:(js + 1) * JM_C][None, :],
                            (P, JM_C)))
        in_maps.append(im)
    return in_maps, inv_gamma


def kernel_run(x, Y_fp, Z_fp, A, bias, act_scale, trace=False, **spmd_kwargs):
    """Build + run on 8 NeuronCores; returns (out, BassKernelResults)."""
    in_maps, inv_gamma = _prepare_inputs(x, Y_fp, Z_fp, A, bias, act_scale)
    nc = _build(inv_gamma)
    res = run_bass_kernel_spmd(
        nc, in_maps, core_ids=list(range(N_CORES)), trace=trace, **spmd_kwargs
    )
    full = np.empty((B_TOT, JM), dtype=np.float32)
    for core in range(N_CORES):
        bs, js = core % B_SHARDS, core // B_SHARDS
        full[bs * B_C:(bs + 1) * B_C, js * JM_C:(js + 1) * JM_C] = (
            res.results[core]["out"].astype(np.float32)
        )
    out = full.reshape(x.shape[0], x.shape[1], JM).astype(x.dtype, copy=False)
    return out, res


def kernel(x, Y_fp, Z_fp, A, bias, act_scale):
    x = np.asarray(x)
    Y_fp = np.asarray(Y_fp)
    Z_fp = np.asarray(Z_fp)
    A = np.asarray(A)
    bias = np.asarray(bias)
    act_scale = np.asarray(act_scale)
    out, _ = kernel_run(x, Y_fp, Z_fp, A, bias, act_scale, trace=False)
    return out


# revision 28
# speedup vs baseline: 1.2049x; 1.0188x over previous
# Trainium2 Bass kernel for nn_BQQLinear (quantized bilinear linear layer).
#
# Math: the reference collapses exactly to
#     out[b, (j,m)] = quant8(x)[b, (k,n)] @ W[(k,n), (j,m)] + bias[(j,m)]
# where W folds the 1-bit-quantized Y/Z factors and the A-correction terms
# (see _fold_weights). W is a pure function of the small weight tensors ->
# folded on host at load time. The activation quant is elementwise host
# prep; the 2048x1024x1024 matmul + bias runs on the NeuronCores.
#
# Sharding: 4-way over batch x 2-way over output columns. Per core:
# [512, 1024] @ [1024, 512] + bias.
#
# PE strategy: fp8e4m3 DoubleRow matmuls (2 ifmap columns/cycle => 256-deep
# contraction per instruction at the same 216ns as a normal 512-wide
# matmul). W is split W ~= H + R with both terms e4m3 at the SAME scale,
# accumulated into the same PSUM bank; R is applied to 6 of the 8 k-tiles
# (the two k-tiles with the smallest residual energy are permuted into the
# first k-pair and skip R). 28 matmul instructions instead of 32, rel err
# ~1.44e-2 (vs 2e-2 gate).
#
# Schedule: DMA issues spread over scalar/sync/vector engines in parallel
# (each DIRECT2D costs ~650ns of sequencer time), first chunks small so
# the real matmul stream starts ~2us into the exec window, k-pair-outer /
# bank-inner order tracks the W stream, last two k-pairs bank-major so
# PSUM banks finish staggered and the DVE evict + out-DMA pipeline
# overlaps the matmul tail. A PE warm spinner keeps the clock ramped from
# window start until the first chunk lands.

import numpy as np
import ml_dtypes

import concourse.bacc as bacc
import concourse.mybir as mybir
import concourse.tile as tile
from concourse.bass import ts
from concourse.bass_utils import run_bass_kernel_spmd

N_CORES = 8
P = 128
KN = 1024                # k*n contraction dim
JM = 1024                # j*m output dim
B_TOT = 2048             # flattened batch
B_SHARDS = 4
C_SHARDS = 2
B_C = B_TOT // B_SHARDS      # 512 rows per core
JM_C = JM // C_SHARDS        # 512 cols per core
B_TILES = B_C // P           # 4
K_TILES = KN // P            # 8
KP = K_TILES // 2            # 4 k-pairs (DoubleRow contracts 2 k-tiles)
R_PAIRS = (0, 1, 2)          # k-pairs that get the residual term
QMAX = 127.0
F8_SAT = 112.0               # e4m3 grid point; q=+-127 maps here exactly
F8_DT = mybir.dt.float8e4
F8_NP = ml_dtypes.float8_e4m3
W_AMAX = 8.0                 # |W*gamma| target max
MM_DT = mybir.dt.float16
DR = mybir.MatmulPerfMode.DoubleRow
WARM_NS = [512] * 6 + [128] * 2   # spinner MM sizes
# chunking: per-k-pair x and W tensors, DMAs issued in need order
# alternating between the two HWDGE rings (sync: W, scalar: x) so the
# shared queues serve chunks in consumption order. Each chunk is its own
# DRAM tensor so every DMA is fully contiguous.


def _fold_weights(Y_fp, Z_fp, A, dtype=np.float64):
    """Fold the quantized factorization into a single [KN, JM] weight."""
    Y = Y_fp.astype(dtype)
    Z = Z_fp.astype(dtype)
    Af = A.astype(dtype)
    p, j, k, m, l = Y.shape
    n = Z.shape[-1]

    Y_scale = np.mean(np.abs(Y), axis=(-2, -1), keepdims=True)
    Z_scale = np.mean(np.abs(Z), axis=(-2, -1), keepdims=True)
    Y_q = np.abs(Y_scale) * np.sign(Y)          # (p,j,k,m,l)
    Z_q = np.abs(Z_scale) * np.sign(Z)          # (p,j,k,l,n)

    # out1: sum_{p,l} A0 * Y_q * Z_q  -> [k,n,j,m]
    W = np.einsum('pjk,pjkml,pjkln->knjm', Af[..., 0], Y_q, Z_q, optimize=True)
    # out2: B_coef[j,k,m] = sum_p A1 * sum_l Y_q ; X enters via Sx (sum over n)
    B_coef = np.einsum('pjk,pjkm->jkm', Af[..., 1], Y_q.sum(-1))
    W += B_coef.transpose(1, 0, 2)[:, None, :, :]
    # out3: C_coef[j,k,n] = sum_p A2 * sum_l Z_q ; broadcast over m
    C_coef = np.einsum('pjk,pjkn->jkn', Af[..., 2], Z_q.sum(-2))
    W += C_coef.transpose(1, 2, 0)[:, :, :, None]
    # out4: D_coef[j,k] = sum_p A3 ; broadcast over n, m
    W += Af[..., 3].sum(0).T[:, None, :, None]
    return W.reshape(k * n, j * m)


def _build(inv_gamma):
    """Per-core Tile kernel: [B_C,KN] @ [KN,JM_C] + bias, evict on DVE."""
    nc = bacc.Bacc(
        "TRN2", target_bir_lowering=False, debug=False,
        enable_asserts=False, num_devices=N_CORES,
        enable_partition_id=False,
    )
    # x: kp0, kp1 separate (early, small), kp2+3 combined; W: per-kp H+R
    # (kp3 H only). 8 input transfers total — few enough to dodge the
    # per-transfer completion tax, fine enough for an early MM start.
    xt0 = nc.dram_tensor("xt0", [P, 2 * B_C], F8_DT,
                         kind="ExternalInput").ap()
    xt1 = nc.dram_tensor("xt1", [P, 2 * B_C], F8_DT,
                         kind="ExternalInput").ap()
    xt23 = nc.dram_tensor("xt23", [P, 2 * 2 * B_C], F8_DT,
                          kind="ExternalInput").ap()
    wts = [nc.dram_tensor(f"wt{c}", [P, 4 * JM_C], F8_DT,
                          kind="ExternalInput").ap() for c in (0, 1, 2)]
    wh3 = nc.dram_tensor("wh3", [P, 2 * JM_C], F8_DT,
                         kind="ExternalInput").ap()
    bi = nc.dram_tensor("bi", [P, JM_C], MM_DT, kind="ExternalInput").ap()
    out = nc.dram_tensor("out", [B_C, JM_C], MM_DT, kind="ExternalOutput").ap()

    out_t = out.rearrange("(bt p) j -> bt p j", p=P)

    with tile.TileContext(nc) as tc:
        with (
            tc.tile_pool(name="sb", bufs=1) as sb,
            tc.tile_pool(name="ps", bufs=1, space="PSUM") as ps,
        ):
            # --- warm-tile memset first: the PE spinner is gated on it ---
            warm_sb = sb.tile([P, 512], MM_DT, tag="warm")
            nc.gpsimd.memset(warm_sb[:, 0:1], 0.0)

            # --- input DMAs: need-order, alternating rings ---
            x0_sb = sb.tile([P, 2, B_C], F8_DT, tag="x0", name="x0")
            x1_sb = sb.tile([P, 2, B_C], F8_DT, tag="x1", name="x1")
            x23_sb = sb.tile([P, 2, 2, B_C], F8_DT, tag="x23", name="x23")
            w_sb = [sb.tile([P, 2, 2, JM_C], F8_DT, tag=f"w{c}", name=f"w{c}")
                    for c in (0, 1, 2)]
            wh3_sb = sb.tile([P, 2, JM_C], F8_DT, tag="wh3", name="wh3")
            bias_sb = sb.tile([P, JM_C], MM_DT, tag="bias")

            two = lambda t: t.rearrange("p (two b) -> p two b", two=2)
            hr2 = lambda t: t.rearrange("p (hr two j) -> p hr two j",
                                        hr=2, two=2)
            nc.scalar.dma_start(x0_sb[:], two(xt0))
            nc.sync.dma_start(w_sb[0][:], hr2(wts[0]))
            nc.scalar.dma_start(x1_sb[:], two(xt1))
            nc.sync.dma_start(w_sb[1][:], hr2(wts[1]))
            nc.scalar.dma_start(x23_sb[:], xt23.rearrange(
                "p (kp two b) -> p kp two b", kp=2, two=2))
            nc.sync.dma_start(w_sb[2][:], hr2(wts[2]))
            nc.sync.dma_start(wh3_sb[:], two(wh3))
            nc.scalar.dma_start(bias_sb[:], bi[:])

            # --- PE warm spinner (results unused): keeps the clock ramping
            # while the first chunks land
            warm_psum = ps.tile([1, 512], mybir.dt.float32, tag="pswarm")
            for n in WARM_NS:
                nc.tensor.matmul(
                    warm_psum[:, :n], lhsT=warm_sb[:, 0:1], rhs=warm_sb[:, :n],
                    start=True, stop=True,
                )

            psum = {
                bt: ps.tile([P, JM_C], mybir.dt.float32, tag=f"ps{bt}",
                            name=f"ps{bt}")
                for bt in range(B_TILES)
            }

            def x_ap(kp, bt):
                if kp == 0:
                    return x0_sb[:, :, ts(bt, P)]
                if kp == 1:
                    return x1_sb[:, :, ts(bt, P)]
                return x23_sb[:, kp - 2, :, ts(bt, P)]

            def mm(kp, bt, hr, start, stop):
                rhs = wh3_sb[:] if kp == 3 else w_sb[kp][:, hr]
                nc.tensor.matmul(
                    psum[bt][:], lhsT=x_ap(kp, bt), rhs=rhs,
                    start=start, stop=stop, perf_mode=DR,
                )

            # kp0, kp1 (H,R): k-outer, bank-inner -> tracks the chunk
            # stream
            for bt in range(B_TILES):
                mm(0, bt, 0, True, False)
                mm(0, bt, 1, False, False)
            for bt in range(B_TILES):
                mm(1, bt, 0, False, False)
                mm(1, bt, 1, False, False)
            # kp2+kp3: bank-major so banks finish staggered for the
            # evict + out-DMA pipeline
            NH = JM_C // 2
            o_sb = {}
            for bt in range(B_TILES):
                mm(2, bt, 0, False, False)
                mm(2, bt, 1, False, False)
                mm(3, bt, 0, False, True)
                o_sb[bt] = sb.tile([P, JM_C], MM_DT, tag=f"o{bt}",
                                   name=f"o{bt}")
                if bt < B_TILES - 1:
                    # out = psum * inv_gamma + bias
                    nc.vector.scalar_tensor_tensor(
                        o_sb[bt][:], psum[bt][:], inv_gamma, bias_sb[:],
                        mybir.AluOpType.mult, mybir.AluOpType.add,
                    )
                    eng = nc.sync if bt % 2 == 0 else nc.scalar
                    eng.dma_start(out_t[bt][:], o_sb[bt][:])
                else:
                    # last bank: halve the exposed evict->dma chain, one
                    # half per ring
                    for h in range(2):
                        nc.vector.scalar_tensor_tensor(
                            o_sb[bt][:, ts(h, NH)], psum[bt][:, ts(h, NH)],
                            inv_gamma, bias_sb[:, ts(h, NH)],
                            mybir.AluOpType.mult, mybir.AluOpType.add,
                        )
                        eng2 = nc.sync if h == 0 else nc.scalar
                        eng2.dma_start(out_t[bt][:, ts(h, NH)],
                                       o_sb[bt][:, ts(h, NH)])

            # keep the warm matmuls live (guard against DCE)
            sink = sb.tile([1, 1], mybir.dt.float32, tag="sink")
            nc.vector.tensor_copy(out=sink[:], in_=warm_psum[0:1, 0:1])

    nc.compile()
    return nc


def _prepare_inputs(x, Y_fp, Z_fp, A, bias, act_scale):
    s = max(abs(float(np.asarray(act_scale).reshape(-1)[0])), 1e-8)
    # activation quant -> integer codes, then e4m3 with saturation at F8_SAT
    q = np.clip(np.rint(x.astype(np.float32).reshape(B_TOT, KN)
                        / np.float32(s)), -QMAX, QMAX)
    qf8 = (q * np.float32(F8_SAT / QMAX)).astype(F8_NP)

    # fold everything else into W; quantize W*gamma ~= H + R (both e4m3,
    # same scale -> accumulate in one PSUM group)
    W = _fold_weights(Y_fp, Z_fp, A)
    W_s = W * (s * QMAX / F8_SAT)
    gamma = W_AMAX / np.abs(W_s).max()
    Wg = (W_s * gamma).astype(np.float32)
    H = Wg.astype(F8_NP)
    R = (Wg - H.astype(np.float32)).astype(F8_NP)
    inv_gamma = float(1.0 / np.float32(gamma))

    # permute k-tiles so the two lowest-residual tiles form the last
    # k-pair (which skips R)
    resid = ((Wg - H.astype(np.float32)) ** 2).reshape(K_TILES, P, JM)
    order = np.argsort(resid.sum(axis=(1, 2)))            # ascending
    perm = np.concatenate([np.sort(order[2:]), order[:2]])
    H8 = H.reshape(K_TILES, P, JM)[perm]                  # [8,128,JM]
    R8 = R.reshape(K_TILES, P, JM)[perm]
    qT = np.ascontiguousarray(qf8.T).reshape(K_TILES, P, B_TOT)[perm]

    bias16 = np.asarray(bias, dtype=np.float16)

    in_maps = []
    for core in range(N_CORES):
        bs, js = core % B_SHARDS, core // B_SHARDS
        xk = qT[:, :, bs * B_C:(bs + 1) * B_C]            # [8,128,B_C]
        xp = xk.reshape(KP, 2, P, B_C)
        Hc = H8[:, :, js * JM_C:(js + 1) * JM_C]          # [8,128,JM_C]
        Rc = R8[:, :, js * JM_C:(js + 1) * JM_C]
        Hp = Hc.reshape(KP, 2, P, JM_C)
        Rp = Rc.reshape(KP, 2, P, JM_C)

        def pt(a):                                        # [2,P,F] -> [P,2F]
            return np.ascontiguousarray(a.transpose(1, 0, 2)).reshape(
                P, -1)

        im = {
            "xt0": pt(xp[0]),
            "xt1": pt(xp[1]),
            "xt23": np.ascontiguousarray(
                xp[2:4].transpose(2, 0, 1, 3)).reshape(P, 2 * 2 * B_C),
            "wh3": pt(Hp[3]),
        }
        for kp in (0, 1, 2):
            blk = np.stack([Hp[kp], Rp[kp]], axis=0)      # [2(hr),2,P,JM_C]
            im[f"wt{kp}"] = np.ascontiguousarray(
                blk.transpose(2, 0, 1, 3)).reshape(P, 4 * JM_C)
        im["bi"] = np.ascontiguousarray(
            np.broadcast_to(bias16[js * JM_C:(js + 1) * JM_C][None, :],
                            (P, JM_C)))
        in_maps.append(im)
    return in_maps, inv_gamma


def kernel_run(x, Y_fp, Z_fp, A, bias, act_scale, trace=False, **spmd_kwargs):
    """Build + run on 8 NeuronCores; returns (out, BassKernelResults)."""
    in_maps, inv_gamma = _prepare_inputs(x, Y_fp, Z_fp, A, bias, act_scale)
    nc = _build(inv_gamma)
    res = run_bass_kernel_spmd(
        nc, in_maps, core_ids=list(range(N_CORES)), trace=trace, **spmd_kwargs
    )
    full = np.empty((B_TOT, JM), dtype=np.float32)
    for core in range(N_CORES):
        bs, js = core % B_SHARDS, core // B_SHARDS
        full[bs * B_C:(bs + 1) * B_C, js * JM_C:(js + 1) * JM_C] = (
            res.results[core]["out"].astype(np.float32)
        )
    out = full.reshape(x.shape[0], x.shape[1], JM).astype(x.dtype, copy=False)
    return out, res


def kernel(x, Y_fp, Z_fp, A, bias, act_scale):
    x = np.asarray(x)
    Y_fp = np.asarray(Y_fp)
    Z_fp = np.asarray(Z_fp)
    A = np.asarray(A)
    bias = np.asarray(bias)
    act_scale = np.asarray(act_scale)
    out, _ = kernel_run(x, Y_fp, Z_fp, A, bias, act_scale, trace=False)
    return out
